# revision 12
# baseline (speedup 1.0000x reference)
"""Causal MHA (B=4, T=2048, D=1024, H=16, Dh=64) on 8 TRN2 NeuronCores.

Sharding: tensor-parallel over heads (2 groups of 8 heads; W_q/W_k/W_v split
column-wise, W_o row-wise) x data-parallel over batch (4 batches). Core
c = (b, g) computes a partial output x[b] attention with head-group g; the
host sums the two head-group partials per batch.

All device matmuls run in bf16 (fp32 PSUM accumulation); verified numerically
at ~4e-3 rel err vs the fp32 reference (tolerance 2e-2).

Host-side prep per core: x[b] is transposed (xT [D,T]) and cast to bf16 so the
kernel needs no on-device transposes; weights cast to bf16.

Per-core kernel (Bass/Tile):
  B: Q^T, K^T [I,T] bf16 (heads pair-interleaved per 128-row chunk), V stored
     per-head [128,h,65] bf16 with a ones column for the softmax denominator.
     Emitted per 512-wide t-block, interleaved with attention q-blocks.
  C: per head pair p (rows 0-63 / 64-127 of chunk p) and q-block of 512:
     S^T[k,q] for both heads back-to-back (row-tiled halves of the PE array,
     bf16 PSUM out, both heads in one bank), one exp per k-tile covering both
     heads (scale folded), lower-tri mask multiply on diagonal tiles, then
     ctx^T[65,q] accumulates V_aug^T P^T in PSUM; row 64 is the denominator.
     Normalize via DVE reciprocal + ones-matmul partition broadcast.
  D: out = ctx^T.T Wo accumulated over inner chunks, DMA PSUM -> SBUF -> DRAM,
     emitted per q-block as soon as all pairs finish it.
"""

import numpy as np
import ml_dtypes

import concourse.bass as bass
import concourse.mybir as mybir
import concourse.tile as tile
from concourse import bacc
from concourse.bass_utils import run_bass_kernel_spmd

B, T, D = 4, 2048, 1024
H_TOT, DH = 16, 64
N_CORES = 8
HPC = 8                  # heads per core
NPAIR = HPC // 2         # head pairs per core (= 128-row chunks of I)
I = HPC * DH             # 512: inner width per core
F32 = mybir.dt.float32
F32R = mybir.dt.float32r
BF16 = mybir.dt.bfloat16
SCALE = float(DH) ** -0.5
QB = 512                 # q-block width
NQB = T // QB            # 4 q-blocks
NTT = T // 128           # 16 t-tiles

_NC_CACHE = []


def _emit(nc, tc, ctx):
    xT_d = nc.dram_tensor("xt", [D, T], BF16, kind="ExternalInput")
    wq_d = nc.dram_tensor("wq", [D, I], BF16, kind="ExternalInput")
    wk_d = nc.dram_tensor("wk", [D, I], BF16, kind="ExternalInput")
    wv_d = nc.dram_tensor("wv", [D, I], BF16, kind="ExternalInput")
    wo_d = nc.dram_tensor("wo", [I, D], BF16, kind="ExternalInput")
    o_d = nc.dram_tensor("o", [T, D], F32, kind="ExternalOutput")

    xT_view = xT_d.ap().rearrange("(c p) t -> p c t", p=128)   # [128,8,2048]
    o_view = o_d.ap().rearrange("(n p) d -> n p d", p=128)     # [16,128,1024]

    persist = ctx.enter_context(tc.tile_pool(name="persist", bufs=1))

    # constant: lower-tri keep mask (bf16)
    ltri32 = persist.tile([128, 128], F32, tag="ltri32")
    nc.gpsimd.memset(ltri32[:], 1.0)
    nc.gpsimd.affine_select(
        out=ltri32[:], in_=ltri32[:], compare_op=mybir.AluOpType.is_ge,
        fill=0.0, base=0, pattern=[[1, 128]], channel_multiplier=-1,
    )
    ltri = persist.tile([128, 2, 128], BF16, tag="ltri")
    nc.vector.tensor_copy(ltri[:, 0, :], ltri32[:])
    nc.vector.tensor_copy(ltri[:, 1, :], ltri32[:])
    onescol32 = persist.tile([128, HPC, 1], F32, tag="onescol32")
    nc.gpsimd.memset(onescol32[:], 1.0)

    # persistent SBUF tensors
    xT = persist.tile([128, 8, T], BF16, tag="xT")              # [D-chunk, T]
    wq_t = persist.tile([128, 8, I], BF16, tag="wq")
    wk_t = persist.tile([128, 8, I], BF16, tag="wk")
    wv_t = persist.tile([128, 8, I], BF16, tag="wv")
    wo_t = persist.tile([128, 4, D], BF16, tag="wo")
    qT = persist.tile([128, NPAIR, T], BF16, tag="qT")          # I-chunk major
    kT = persist.tile([128, NPAIR, T], BF16, tag="kT")
    v3 = persist.tile([128, NTT, HPC, DH + 1], BF16, tag="v3")
    ctxT = persist.tile([128, NPAIR, T], BF16, tag="ctxT")

    # weight + x loads (x in 4 t-quarters so stage B can start early)
    for w_t, w_d in ((wq_t, wq_d), (wk_t, wk_d), (wv_t, wv_d)):
        nc.sync.dma_start(w_t[:], w_d.ap().rearrange("(c p) i -> p c i", p=128))
    nc.sync.dma_start(wo_t[:], wo_d.ap().rearrange("(c p) d -> p c d", p=128))
    for tb in range(NQB):
        nc.sync.dma_start(xT[:, :, tb * QB:(tb + 1) * QB],
                          xT_view[:, :, tb * QB:(tb + 1) * QB])

    # PSUM bank budget (8): qkv/proj 2 + scores 2x2 + ctx 2 = 8
    psum_qkv = ctx.enter_context(tc.tile_pool(name="psum_qkv", bufs=2, space="PSUM"))
    psum_sc = ctx.enter_context(tc.tile_pool(name="psum_sc", bufs=2, space="PSUM"))
    psum_ctx = ctx.enter_context(tc.tile_pool(name="psum_ctx", bufs=2, space="PSUM"))

    ptpool = ctx.enter_context(tc.tile_pool(name="pt", bufs=3))
    recpool = ctx.enter_context(tc.tile_pool(name="rec", bufs=2))
    bcspool = ctx.enter_context(tc.tile_pool(name="bcs", bufs=2))
    outpool = ctx.enter_context(tc.tile_pool(name="out_sb", bufs=3))

    def stage_b(tb):
        """QKV for t-block tb (512 wide)."""
        t0 = tb * QB
        # Q^T / K^T chunks: out rows = I-chunk ic (head pair ic), cols = t
        for ic in range(NPAIR):
            for w_t, dstT in ((wq_t, qT), (wk_t, kT)):
                ps = psum_qkv.tile([128, QB], F32, tag="qkv")
                for dc in range(8):
                    nc.tensor.matmul(
                        ps[:],
                        w_t[:, dc, ic * 128:(ic + 1) * 128],
                        xT[:, dc, t0:t0 + QB],
                        start=(dc == 0), stop=(dc == 7),
                    )
                with nc.allow_low_precision(reason="bf16 storage of Q/K"):
                    nc.vector.tensor_copy(dstT[:, ic, t0:t0 + QB], ps[:])
        # V natural per t-tile, per-head columns + ones column
        for tt in range(4 * tb, 4 * tb + 4):
            ps = psum_qkv.tile([128, I], F32, tag="qkv")
            for dc in range(8):
                nc.tensor.matmul(
                    ps[:],
                    xT[:, dc, tt * 128:(tt + 1) * 128],
                    wv_t[:, dc, :],
                    start=(dc == 0), stop=(dc == 7),
                )
            with nc.allow_low_precision(reason="bf16 storage of V"):
                nc.vector.tensor_copy(
                    v3[:, tt, :, 0:DH],
                    ps[:].rearrange("p (h d) -> p h d", h=HPC),
                )
            nc.vector.tensor_copy(v3[:, tt, :, DH:DH + 1], onescol32[:])

    def attention(qb):
        """All head pairs for q-block qb."""
        q0 = qb * QB
        n_kt = 4 * (qb + 1)
        for p in range(NPAIR):
            cps = [psum_ctx.tile([DH + 1, QB], F32, tag="ctx", name=f"cps{qb}_{p}_{i}")
                   for i in range(2)]
            for kt in range(n_kt):
                k0 = kt * 128
                m = kt - 4 * qb  # >= 0: this k-tile touches the diagonal
                c0 = max(m, 0) * 128
                # scores for both heads, row-tiled halves of the PE array
                sc = psum_sc.tile([128, 2, QB], F32, tag="sc")
                for hl in range(2):
                    po = hl * 64
                    nc.tensor.matmul(
                        sc[:, hl, c0:QB],
                        kT[po:po + 64, p, k0:k0 + 128],
                        qT[po:po + 64, p, q0 + c0:q0 + QB],
                        start=True, stop=True,
                    )
                pt = ptpool.tile([128, 2, QB], BF16, tag="pt")
                nc.scalar.activation(
                    pt[:, :, c0:QB], sc[:, :, c0:QB],
                    mybir.ActivationFunctionType.Exp, scale=SCALE,
                )
                if m >= 0:
                    nc.vector.tensor_mul(
                        pt[:, :, c0:c0 + 128],
                        pt[:, :, c0:c0 + 128],
                        ltri[:],
                    )
                for hl in range(2):
                    nc.tensor.matmul(
                        cps[hl][:, c0:QB], v3[:, kt, 2 * p + hl, :],
                        pt[:, hl, c0:QB],
                        start=(kt == 0), stop=(kt == n_kt - 1),
                    )
            for hl in range(2):
                po = hl * 64
                rec = recpool.tile([1, QB], F32, tag="rec")
                nc.vector.reciprocal(rec[:], cps[hl][DH:DH + 1, :])
                bcs = bcspool.tile([64, QB], F32, tag="bcs")
                nc.gpsimd.partition_broadcast(bcs[:], rec[:])
                with nc.allow_low_precision(reason="bf16 storage of ctx"):
                    nc.vector.tensor_mul(
                        ctxT[po:po + 64, p, q0:q0 + QB], cps[hl][0:DH, :], bcs[:])

    def stage_d(qb):
        """Output projection for the t-tiles of q-block qb."""
        for tt in range(4 * qb, 4 * qb + 4):
            osb = outpool.tile([128, D], F32, tag="osb")
            for db in range(2):
                ops = psum_qkv.tile([128, 512], F32, tag="qkv")
                for ic in range(NPAIR):
                    nc.tensor.matmul(
                        ops[:],
                        ctxT[:, ic, tt * 128:(tt + 1) * 128],
                        wo_t[:, ic, db * 512:(db + 1) * 512],
                        start=(ic == 0), stop=(ic == NPAIR - 1),
                    )
                nc.vector.tensor_copy(osb[:, db * 512:(db + 1) * 512], ops[:])
            nc.sync.dma_start(o_view[tt], osb[:])

    for blk in range(NQB):
        stage_b(blk)
        attention(blk)
        stage_d(blk)


def _build():
    from contextlib import ExitStack

    nc = bacc.Bacc("TRN2", target_bir_lowering=False, debug=False,
                   enable_asserts=True, num_devices=N_CORES)
    with tile.TileContext(nc) as tc:
        with ExitStack() as ctx:
            _emit(nc, tc, ctx)
    nc.compile()
    return nc


def _get_nc():
    if not _NC_CACHE:
        _NC_CACHE.append(_build())
    return _NC_CACHE[0]


def _in_maps(x, W_q, W_k, W_v, W_o):
    bf = ml_dtypes.bfloat16
    maps = []
    xts = [np.ascontiguousarray(x[b].T).astype(bf) for b in range(B)]
    for c in range(N_CORES):
        b, g = c // 2, c % 2
        s = slice(g * I, (g + 1) * I)
        maps.append({
            "xt": xts[b],
            "wq": np.ascontiguousarray(W_q[:, s]).astype(bf),
            "wk": np.ascontiguousarray(W_k[:, s]).astype(bf),
            "wv": np.ascontiguousarray(W_v[:, s]).astype(bf),
            "wo": np.ascontiguousarray(W_o[s, :]).astype(bf),
        })
    return maps


def kernel(**inputs):
    x = np.asarray(inputs["x"], dtype=np.float32)
    W_q = np.asarray(inputs["W_q"], dtype=np.float32)
    W_k = np.asarray(inputs["W_k"], dtype=np.float32)
    W_v = np.asarray(inputs["W_v"], dtype=np.float32)
    W_o = np.asarray(inputs["W_o"], dtype=np.float32)

    nc = _get_nc()
    res = run_bass_kernel_spmd(nc, _in_maps(x, W_q, W_k, W_v, W_o),
                               core_ids=list(range(N_CORES)))
    out = np.empty((B, T, D), dtype=np.float32)
    for b in range(B):
        out[b] = res.results[2 * b]["o"] + res.results[2 * b + 1]["o"]
    return out


# revision 24
# speedup vs baseline: 94.9105x; 94.9105x over previous
"""Causal MHA (B=4, T=2048, D=1024, H=16, Dh=64) on 8 TRN2 NeuronCores.

Sharding: tensor-parallel over heads (2 groups of 8 heads; W_q/W_k/W_v split
column-wise, W_o row-wise) x data-parallel over batch (4 batches). Core
c = (b, g) computes a partial output x[b] attention with head-group g; the
host sums the two head-group partials per batch.

All device matmuls run in bf16 (fp32 PSUM accumulation); verified numerically
at ~4e-3 rel err vs the fp32 reference (tolerance 2e-2).

Host-side prep per core: x[b] is transposed (xT [D,T]) and cast to bf16 so the
kernel needs no on-device transposes; weights cast to bf16.

Per-core kernel (Bass/Tile):
  B: Q^T, K^T [I,T] bf16 (heads pair-interleaved per 128-row chunk), V stored
     per-head [128,h,65] bf16 with a ones column for the softmax denominator.
     Emitted per 512-wide t-block, interleaved with attention q-blocks.
  C: per head pair p (rows 0-63 / 64-127 of chunk p) and q-block of 512:
     S^T[k,q] for both heads back-to-back (row-tiled halves of the PE array,
     bf16 PSUM out, both heads in one bank), one exp per k-tile covering both
     heads (scale folded), lower-tri mask multiply on diagonal tiles, then
     ctx^T[65,q] accumulates V_aug^T P^T in PSUM; row 64 is the denominator.
     Normalize via DVE reciprocal + ones-matmul partition broadcast.
  D: out = ctx^T.T Wo accumulated over inner chunks, DMA PSUM -> SBUF -> DRAM,
     emitted per q-block as soon as all pairs finish it.
"""

import numpy as np
import ml_dtypes

import concourse.bass as bass
import concourse.mybir as mybir
import concourse.tile as tile
from concourse import bacc
from concourse.bass_utils import run_bass_kernel_spmd

B, T, D = 4, 2048, 1024
H_TOT, DH = 16, 64
N_CORES = 8
HPC = 8                  # heads per core
NPAIR = HPC // 2         # head pairs per core (= 128-row chunks of I)
I = HPC * DH             # 512: inner width per core
F32 = mybir.dt.float32
F32R = mybir.dt.float32r
BF16 = mybir.dt.bfloat16
SCALE = float(DH) ** -0.5
QB = 512                 # q-block width
NQB = T // QB            # 4 q-blocks
NTT = T // 128           # 16 t-tiles

_NC_CACHE = []


def _emit(nc, tc, ctx):
    xT_d = nc.dram_tensor("xt", [D, T], BF16, kind="ExternalInput")
    wq_d = nc.dram_tensor("wq", [D, I], BF16, kind="ExternalInput")
    wk_d = nc.dram_tensor("wk", [D, I], BF16, kind="ExternalInput")
    wv_d = nc.dram_tensor("wv", [D, I], BF16, kind="ExternalInput")
    wo_d = nc.dram_tensor("wo", [I, D], BF16, kind="ExternalInput")
    o_d = nc.dram_tensor("o", [T, D], F32, kind="ExternalOutput")

    xT_view = xT_d.ap().rearrange("(c p) t -> p c t", p=128)   # [128,8,2048]
    o_view = o_d.ap().rearrange("(n p) d -> n p d", p=128)     # [16,128,1024]

    persist = ctx.enter_context(tc.tile_pool(name="persist", bufs=1))

    # constant: lower-tri keep mask (bf16)
    ltri32 = persist.tile([128, 128], F32, tag="ltri32")
    nc.gpsimd.memset(ltri32[:], 1.0)
    nc.gpsimd.affine_select(
        out=ltri32[:], in_=ltri32[:], compare_op=mybir.AluOpType.is_ge,
        fill=0.0, base=0, pattern=[[1, 128]], channel_multiplier=-1,
    )
    ltri = persist.tile([128, 2, 128], BF16, tag="ltri")
    nc.vector.tensor_copy(ltri[:, 0, :], ltri32[:])
    nc.vector.tensor_copy(ltri[:, 1, :], ltri32[:])
    onescol32 = persist.tile([128, HPC, 1], F32, tag="onescol32")
    nc.gpsimd.memset(onescol32[:], 1.0)

    # persistent SBUF tensors
    xT = persist.tile([128, 8, T], BF16, tag="xT")              # [D-chunk, T]
    wq_t = persist.tile([128, 8, I], BF16, tag="wq")
    wk_t = persist.tile([128, 8, I], BF16, tag="wk")
    wv_t = persist.tile([128, 8, I], BF16, tag="wv")
    wo_t = persist.tile([128, 4, D], BF16, tag="wo")
    qT = persist.tile([128, NPAIR, T], BF16, tag="qT")          # I-chunk major
    kT = persist.tile([128, NPAIR, T], BF16, tag="kT")
    v3 = persist.tile([128, NTT, HPC, DH + 1], BF16, tag="v3")
    ctxT = persist.tile([128, NPAIR, T], BF16, tag="ctxT")

    # weight + x loads, ordered to minimize time-to-first-matmul: x-quarter0,
    # then the wq slice the first Q^T chain needs, then the rest.
    def load_x_quarter(tb):
        nc.sync.dma_start(xT[:, :, tb * QB:(tb + 1) * QB],
                          xT_view[:, :, tb * QB:(tb + 1) * QB])
    wq_view = wq_d.ap().rearrange("(c p) i -> p c i", p=128)
    load_x_quarter(0)
    nc.sync.dma_start(wq_t[:, :, 0:128], wq_view[:, :, 0:128])
    nc.sync.dma_start(wq_t[:, :, 128:I], wq_view[:, :, 128:I])
    nc.sync.dma_start(wk_t[:], wk_d.ap().rearrange("(c p) i -> p c i", p=128))
    nc.sync.dma_start(wv_t[:], wv_d.ap().rearrange("(c p) i -> p c i", p=128))
    load_x_quarter(1)
    nc.sync.dma_start(wo_t[:], wo_d.ap().rearrange("(c p) d -> p c d", p=128))
    load_x_quarter(2)
    load_x_quarter(3)

    # PSUM bank budget (8): qkv/proj 2 + scores 2x2 + ctx 2 = 8
    psum_qkv = ctx.enter_context(tc.tile_pool(name="psum_qkv", bufs=2, space="PSUM"))
    psum_sc = ctx.enter_context(tc.tile_pool(name="psum_sc", bufs=2, space="PSUM"))
    psum_ctx = ctx.enter_context(tc.tile_pool(name="psum_ctx", bufs=2, space="PSUM"))

    ptpool = ctx.enter_context(tc.tile_pool(name="pt", bufs=3))
    recpool = ctx.enter_context(tc.tile_pool(name="rec", bufs=2))
    bcspool = ctx.enter_context(tc.tile_pool(name="bcs", bufs=2))
    outpool = ctx.enter_context(tc.tile_pool(name="out_sb", bufs=3))

    def stage_b_units(tb):
        """QKV for t-block tb (512 wide); yields after each schedulable unit.

        For tb=0 all Q chains are emitted before the K chains so the PE's
        first ~7us of work only depends on x-quarter0 + the wq DMAs.
        """
        t0 = tb * QB
        # Q^T / K^T chunks: out rows = I-chunk ic (head pair ic), cols = t
        projs = [(wq_t, qT), (wk_t, kT)]
        order = [(ic, pr) for ic in range(NPAIR) for pr in projs]
        for i, (ic, (w_t, dstT)) in enumerate(order):
            ps = psum_qkv.tile([128, QB], F32, tag="qkv")
            for dc in range(8):
                nc.tensor.matmul(
                    ps[:],
                    w_t[:, dc, ic * 128:(ic + 1) * 128],
                    xT[:, dc, t0:t0 + QB],
                    start=(dc == 0), stop=(dc == 7),
                )
            with nc.allow_low_precision(reason="bf16 storage of Q/K"):
                nc.any.tensor_copy(dstT[:, ic, t0:t0 + QB], ps[:])
            if i % 2 == 1:
                yield
        # V natural per t-tile, per-head columns + ones column
        for tt in range(4 * tb, 4 * tb + 4):
            ps = psum_qkv.tile([128, I], F32, tag="qkv")
            for dc in range(8):
                nc.tensor.matmul(
                    ps[:],
                    xT[:, dc, tt * 128:(tt + 1) * 128],
                    wv_t[:, dc, :],
                    start=(dc == 0), stop=(dc == 7),
                )
            with nc.allow_low_precision(reason="bf16 storage of V"):
                nc.vector.tensor_copy(
                    v3[:, tt, :, 0:DH],
                    ps[:].rearrange("p (h d) -> p h d", h=HPC),
                )
            nc.vector.tensor_copy(v3[:, tt, :, DH:DH + 1], onescol32[:])
            yield

    def attention_units(qb, filler=None, every=4):
        """All head pairs for q-block qb; yields after each pair. Drains one
        unit of `filler` every `every` k-tiles so the PE (in-order) always has
        independent matmul work queued between exp-dependent PV chains."""
        q0 = qb * QB
        n_kt = 4 * (qb + 1)
        fill_ctr = 0
        for p in range(NPAIR):
            cps = [psum_ctx.tile([DH + 1, QB], F32, tag="ctx", name=f"cps{qb}_{p}_{i}")
                   for i in range(2)]
            for kt in range(n_kt):
                k0 = kt * 128
                m = kt - 4 * qb  # >= 0: this k-tile touches the diagonal
                c0 = max(m, 0) * 128
                # scores for both heads, row-tiled halves of the PE array
                sc = psum_sc.tile([128, 2, QB], F32, tag="sc")
                for hl in range(2):
                    po = hl * 64
                    nc.tensor.matmul(
                        sc[:, hl, c0:QB],
                        kT[po:po + 64, p, k0:k0 + 128],
                        qT[po:po + 64, p, q0 + c0:q0 + QB],
                        start=True, stop=True,
                    )
                pt = ptpool.tile([128, 2, QB], BF16, tag="pt")
                nc.scalar.activation(
                    pt[:, :, c0:QB], sc[:, :, c0:QB],
                    mybir.ActivationFunctionType.Exp, scale=SCALE,
                )
                if m >= 0:
                    nc.vector.tensor_mul(
                        pt[:, :, c0:c0 + 128],
                        pt[:, :, c0:c0 + 128],
                        ltri[:],
                    )
                for hl in range(2):
                    nc.tensor.matmul(
                        cps[hl][:, c0:QB], v3[:, kt, 2 * p + hl, :],
                        pt[:, hl, c0:QB],
                        start=(kt == 0), stop=(kt == n_kt - 1),
                    )
                fill_ctr += 1
                if filler is not None and fill_ctr % every == 0:
                    drain(filler, 1)
            for hl in range(2):
                po = hl * 64
                rec = recpool.tile([1, QB], F32, tag="rec")
                nc.vector.reciprocal(rec[:], cps[hl][DH:DH + 1, :])
                bcs = bcspool.tile([64, QB], F32, tag="bcs")
                nc.gpsimd.partition_broadcast(bcs[:], rec[:])
                with nc.allow_low_precision(reason="bf16 storage of ctx"):
                    nc.vector.tensor_mul(
                        ctxT[po:po + 64, p, q0:q0 + QB], cps[hl][0:DH, :], bcs[:])
            yield

    def stage_d_units(qb):
        """Output projection for the t-tiles of q-block qb; yields per tile."""
        for tt in range(4 * qb, 4 * qb + 4):
            osb = outpool.tile([128, D], F32, tag="osb")
            for db in range(2):
                ops = psum_qkv.tile([128, 512], F32, tag="qkv")
                for ic in range(NPAIR):
                    nc.tensor.matmul(
                        ops[:],
                        ctxT[:, ic, tt * 128:(tt + 1) * 128],
                        wo_t[:, ic, db * 512:(db + 1) * 512],
                        start=(ic == 0), stop=(ic == NPAIR - 1),
                    )
                nc.vector.tensor_copy(osb[:, db * 512:(db + 1) * 512], ops[:])
            nc.sync.dma_start(o_view[tt], osb[:])
            yield

    def drain(gen, n=None):
        taken = 0
        for _ in gen:
            taken += 1
            if n is not None and taken >= n:
                break

    # Emission: B0 fully, then per q-block interleave attention pairs with
    # filler matmul work for the PE during exp(ACT)-bound stretches. The
    # attention phases grow with qb (causal), so the output projections
    # (D0-D2) are deferred to pad the heaviest phase (qb3); D3 trails.
    drain(stage_b_units(0))
    for qb in range(NQB - 1):
        b_next = stage_b_units(qb + 1)
        for _ in attention_units(qb):
            drain(b_next, 2)
        drain(b_next)
    d_fill = (u for q in range(NQB - 1) for u in stage_d_units(q))
    for _ in attention_units(NQB - 1):
        drain(d_fill, 3)
    drain(d_fill)
    drain(stage_d_units(NQB - 1))


def _build():
    from contextlib import ExitStack

    nc = bacc.Bacc("TRN2", target_bir_lowering=False, debug=False,
                   enable_asserts=True, num_devices=N_CORES)
    with tile.TileContext(nc) as tc:
        with ExitStack() as ctx:
            _emit(nc, tc, ctx)
    nc.compile()
    return nc


def _get_nc():
    if not _NC_CACHE:
        _NC_CACHE.append(_build())
    return _NC_CACHE[0]


def _in_maps(x, W_q, W_k, W_v, W_o):
    bf = ml_dtypes.bfloat16
    maps = []
    xts = [np.ascontiguousarray(x[b].T).astype(bf) for b in range(B)]
    for c in range(N_CORES):
        b, g = c // 2, c % 2
        s = slice(g * I, (g + 1) * I)
        maps.append({
            "xt": xts[b],
            "wq": np.ascontiguousarray(W_q[:, s]).astype(bf),
            "wk": np.ascontiguousarray(W_k[:, s]).astype(bf),
            "wv": np.ascontiguousarray(W_v[:, s]).astype(bf),
            "wo": np.ascontiguousarray(W_o[s, :]).astype(bf),
        })
    return maps


def kernel(**inputs):
    x = np.asarray(inputs["x"], dtype=np.float32)
    W_q = np.asarray(inputs["W_q"], dtype=np.float32)
    W_k = np.asarray(inputs["W_k"], dtype=np.float32)
    W_v = np.asarray(inputs["W_v"], dtype=np.float32)
    W_o = np.asarray(inputs["W_o"], dtype=np.float32)

    nc = _get_nc()
    res = run_bass_kernel_spmd(nc, _in_maps(x, W_q, W_k, W_v, W_o),
                               core_ids=list(range(N_CORES)))
    out = np.empty((B, T, D), dtype=np.float32)
    for b in range(B):
        out[b] = res.results[2 * b]["o"] + res.results[2 * b + 1]["o"]
    return out
